# revision 1
# baseline (speedup 1.0000x reference)
"""CharLSTM Trainium2 kernel: 2-layer LSTM, SEQ=1024, BATCH=64, H=512, E=256, V=512.

Strategy: data-parallel over batch across 8 cores (B=8 per core), identical SPMD
program. Per core:
  - T1 = emb @ Wih1p.T + b1p  [V, 4H] f32 built on device, stored in DRAM.
  - L1 recurrence: per step, gather T1 rows for x_t into SBUF (X1), inject into
    PSUM via identity matmul, accumulate h1T @ Whh1pT (bf16), activations
    (gate-permuted layout i,f,o,g so sigmoid covers f[0:1536]), cell update,
    h1 transposed via PE into h1T ring (stationary for next step + X2 blocks).
  - Lag-block pipeline: L1 block bk runs alongside X2+L2+FC for block bk-1.
  - FC: out = h2T-block.T @ WfcT + bfc -> DMA to output DRAM.

Gate order permuted from PyTorch (i,f,g,o) to (i,f,o,g).
"""

import numpy as np
from contextlib import ExitStack

import concourse.bass as bass
import concourse.bacc as bacc
import concourse.mybir as mybir
import concourse.tile as tile
from concourse.masks import make_identity

F32 = mybir.dt.float32
BF16 = mybir.dt.bfloat16
I32 = mybir.dt.int32

H = 512
E = 256
V = 512
H4 = 4 * H
BS = 16  # steps per block


def build_nc(seq=1024, b=8, num_devices=8):
    nb = seq // BS
    nc = bacc.Bacc("TRN2", target_bir_lowering=False, debug=False,
                   num_devices=num_devices)

    dram = nc.dram_tensor
    x_ext = dram("x", [b, seq], I32, kind="ExternalInput").ap()
    embt_ext = dram("embt", [E, V], BF16, kind="ExternalInput").ap()
    wih1t_ext = dram("wih1t", [E, H4], BF16, kind="ExternalInput").ap()
    b1_ext = dram("b1", [1, H4], F32, kind="ExternalInput").ap()
    whh1t_ext = dram("whh1t", [H, H4], BF16, kind="ExternalInput").ap()
    wih2t_ext = dram("wih2t", [H, H4], BF16, kind="ExternalInput").ap()
    b2_ext = dram("b2", [1, H4], F32, kind="ExternalInput").ap()
    whh2t_ext = dram("whh2t", [H, H4], BF16, kind="ExternalInput").ap()
    wfct_ext = dram("wfct", [H, V], BF16, kind="ExternalInput").ap()
    bfc_ext = dram("bfc", [1, V], F32, kind="ExternalInput").ap()
    h0_ext = dram("h0", [b, H], F32, kind="ExternalInput").ap()
    c0_ext = dram("c0", [b, H], F32, kind="ExternalInput").ap()
    out_ext = dram("out", [seq * b, V], F32, kind="ExternalOutput").ap()
    t1_dram = dram("t1buf", [V, H4], F32).ap()

    with ExitStack() as ctx:
        tc = ctx.enter_context(tile.TileContext(nc))
        # ---- persistent pools ----
        wp = ctx.enter_context(tc.tile_pool(name="weights", bufs=1))
        sp = ctx.enter_context(tc.tile_pool(name="state", bufs=1))
        h1p = ctx.enter_context(tc.tile_pool(name="h1t", bufs=3))
        h2p = ctx.enter_context(tc.tile_pool(name="h2t", bufs=2))
        gp = ctx.enter_context(tc.tile_pool(name="gath", bufs=3))
        ap_ = ctx.enter_context(tc.tile_pool(name="act", bufs=2))
        x2p = ctx.enter_context(tc.tile_pool(name="x2sb", bufs=2))
        op = ctx.enter_context(tc.tile_pool(name="outsb", bufs=2))
        # psum pools: gates 4 banks, tr 1, x2 2, fc 1 -> 8 banks
        pg = ctx.enter_context(tc.tile_pool(name="pg", bufs=1, space="PSUM"))
        ptr = ctx.enter_context(tc.tile_pool(name="ptr", bufs=1, space="PSUM"))
        px2 = ctx.enter_context(tc.tile_pool(name="px2", bufs=1, space="PSUM"))
        pfc = ctx.enter_context(tc.tile_pool(name="pfc", bufs=1, space="PSUM"))

        # ---- load weights ----
        def load4(ext, name):  # [512, N] dram -> [128, 4*N] sbuf (s-major chunks)
            n = ext.shape[1]
            t = wp.tile([128, 4 * n], BF16, tag=name)
            nc.sync.dma_start(
                out=t[:].rearrange("p (s f) -> p s f", s=4),
                in_=ext.rearrange("(s p) f -> p s f", p=128))
            return t

        whh1t_sb = load4(whh1t_ext, "whh1t")
        wih2t_sb = load4(wih2t_ext, "wih2t")
        whh2t_sb = load4(whh2t_ext, "whh2t")
        wfct_sb = load4(wfct_ext, "wfct")

        b1_sb = wp.tile([1, H4], F32, tag="b1")
        nc.sync.dma_start(out=b1_sb[:], in_=b1_ext[:])
        b2_sb = wp.tile([1, H4], F32, tag="b2")
        nc.sync.dma_start(out=b2_sb[:], in_=b2_ext[:])
        bfc_sb = wp.tile([1, V], F32, tag="bfc")
        nc.sync.dma_start(out=bfc_sb[:], in_=bfc_ext[:])

        ones_sb = wp.tile([1, 128], F32, tag="ones")
        nc.gpsimd.memset(ones_sb[:], 1.0)
        id8f = wp.tile([8, 8], F32, tag="id8f")
        make_identity(nc, id8f[:])
        idbf = wp.tile([b, b], BF16, tag="idbf")
        make_identity(nc, idbf[:])

        x_sb = sp.tile([b, seq], I32, tag="x")
        nc.sync.dma_start(out=x_sb[:], in_=x_ext[:])
        c1_sb = sp.tile([b, H], F32, tag="c1")
        nc.sync.dma_start(out=c1_sb[:], in_=c0_ext[:])
        c2_sb = sp.tile([b, H], F32, tag="c2")
        nc.sync.dma_start(out=c2_sb[:], in_=c0_ext[:])

        # ---- T1 = emb @ Wih1p.T + b1p ----
        with tc.tile_pool(name="t1tmp", bufs=1) as t1tmp, \
             tc.tile_pool(name="t1cp", bufs=2) as t1cp:
            embt_sb = t1tmp.tile([128, 2 * V], BF16, tag="embt")
            nc.sync.dma_start(
                out=embt_sb[:].rearrange("p (s f) -> p s f", s=2),
                in_=embt_ext.rearrange("(s p) f -> p s f", p=128))
            wih1t_sb = t1tmp.tile([128, 2 * H4], BF16, tag="wih1t")
            nc.sync.dma_start(
                out=wih1t_sb[:].rearrange("p (s f) -> p s f", s=2),
                in_=wih1t_ext.rearrange("(s p) f -> p s f", p=128))
            for vc in range(4):
                pt = pg.tile([128, H4], F32, tag="pg")
                for fb in range(4):
                    fsl = slice(fb * 512, (fb + 1) * 512)
                    nc.tensor.matmul(pt[:, fsl], lhsT=ones_sb[:, :128],
                                     rhs=b1_sb[:, fsl], start=True, stop=False)
                    for ks in range(2):
                        nc.tensor.matmul(
                            pt[:, fsl],
                            lhsT=embt_sb[:, ks * V + vc * 128: ks * V + (vc + 1) * 128],
                            rhs=wih1t_sb[:, ks * H4 + fb * 512: ks * H4 + (fb + 1) * 512],
                            start=False, stop=(ks == 1))
                t1s = t1cp.tile([128, H4], F32, tag="t1s")
                nc.vector.tensor_copy(t1s[:], pt[:])
                nc.sync.dma_start(out=t1_dram[vc * 128:(vc + 1) * 128, :], in_=t1s[:])

        # ---- initial h transposes ----
        def transpose_h(hbf, dst128x4b):
            """hbf [b, 512] bf16 -> dst [128, 4*b] bf16 (s-major)."""
            ptile = ptr.tile([128, 4 * b], BF16, tag="ptr")
            for s in range(4):
                nc.tensor.transpose(
                    out=ptile[:, s * b:(s + 1) * b],
                    in_=hbf[:, s * 128:(s + 1) * 128],
                    identity=idbf[:])
            nc.vector.tensor_copy(dst128x4b, ptile[:])

        # h1t block tiles: [128, 4 * (BS+1) * b]; slot j in 0..BS, s-plane stride (BS+1)*b
        HTW = (BS + 1) * b  # f-width per s-plane

        def ht_slot(t_tile, j, width=1):
            return t_tile[:].rearrange("p (s q) -> p s q", s=4)[:, :, j * b:(j + width) * b]

        h0_f32 = ap_.tile([b, H], F32, tag="t1_")
        nc.sync.dma_start(out=h0_f32[:], in_=h0_ext[:])
        h0_bf = ap_.tile([b, H], BF16, tag="h1bf")
        nc.vector.tensor_copy(h0_bf[:], h0_f32[:])

        h1blk = h1p.tile([128, 4 * HTW], BF16, tag="h1blk")
        transpose_h(h0_bf, ht_slot(h1blk, BS))
        h2blk = h2p.tile([128, 4 * HTW], BF16, tag="h2blk")
        transpose_h(h0_bf, ht_slot(h2blk, BS))

        def lstm_step(ht_prev_ap, ht_out_ap, c_sb, whht_sb, inject_rhs, gather_idx):
            """One LSTM step. ht_prev_ap: [128, 4, b] stationary (s-plane view).
            inject_rhs: f32 [8, 2048]-like AP for X contribution, or tile to gather into.
            gather_idx: if not None, (idx_ap) -> gather T1 rows into inject tile first.
            Returns new h bf16 tile [b, 512]."""
            gt = pg.tile([b, H4], F32, tag="pg")
            for fb in range(4):
                fsl = slice(fb * 512, (fb + 1) * 512)
                nc.tensor.matmul(gt[:, fsl], lhsT=id8f[:b, :b],
                                 rhs=inject_rhs[:, fsl], start=True, stop=False)
                for s in range(4):
                    nc.tensor.matmul(
                        gt[:, fsl], lhsT=ht_prev_ap[:, s, :],
                        rhs=whht_sb[:, s * H4 + fb * 512: s * H4 + (fb + 1) * 512],
                        start=False, stop=(s == 3))
            sig = ap_.tile([b, 3 * H], F32, tag="sig")
            nc.scalar.activation(sig[:], gt[:, 0:3 * H],
                                 mybir.ActivationFunctionType.Sigmoid)
            tg = ap_.tile([b, H], F32, tag="tg")
            nc.scalar.activation(tg[:], gt[:, 3 * H:4 * H],
                                 mybir.ActivationFunctionType.Tanh)
            t1_ = ap_.tile([b, H], F32, tag="t1_")
            nc.vector.tensor_mul(t1_[:], sig[:, H:2 * H], c_sb[:])      # f*c
            t2_ = ap_.tile([b, H], F32, tag="t2_")
            nc.vector.tensor_mul(t2_[:], sig[:, 0:H], tg[:])            # i*g
            nc.vector.tensor_add(c_sb[:], t1_[:], t2_[:])               # c = f*c+i*g
            tc_ = ap_.tile([b, H], F32, tag="tg")
            nc.scalar.activation(tc_[:], c_sb[:],
                                 mybir.ActivationFunctionType.Tanh)
            hbf = ap_.tile([b, H], BF16, tag="h1bf")
            nc.vector.tensor_mul(hbf[:], sig[:, 2 * H:3 * H], tc_[:])   # h = o*tanh(c)
            transpose_h(hbf, ht_out_ap)
            return hbf

        h1_tiles = {}   # bk -> h1blk tile
        h1_tiles[-1] = h1blk
        h2_prev = h2blk

        def l1_block(bk):
            prev = h1_tiles[bk - 1]
            cur = h1p.tile([128, 4 * HTW], BF16, tag="h1blk")
            # slot 0 of cur = slot BS of prev (copy boundary)
            nc.vector.tensor_copy(ht_slot(cur, 0), ht_slot(prev, BS))
            h1_tiles[bk] = cur
            for j in range(BS):
                t = bk * BS + j
                x1 = gp.tile([b, H4], F32, tag="x1")
                nc.gpsimd.indirect_dma_start(
                    out=x1[:], out_offset=None, in_=t1_dram[:],
                    in_offset=bass.IndirectOffsetOnAxis(ap=x_sb[:, t:t + 1], axis=0))
                lstm_step(ht_slot(cur, j), ht_slot(cur, j + 1), c1_sb,
                          whh1t_sb, x1[:], None)
            if bk >= 2 and (bk - 2) in h1_tiles:
                del h1_tiles[bk - 2]

        def l2_block(bk):
            nonlocal h2_prev
            h1 = h1_tiles[bk]
            # X2 = h1(block).T-stored @ Wih2pT + b2 : out [(BS*b)=128, 2048]
            x2 = x2p.tile([BS * b, H4], F32, tag="x2")
            for half in range(2):
                pt = px2.tile([128, 1024], F32, tag="px2")
                for fb in range(2):
                    fo = half * 1024 + fb * 512
                    psl = slice(fb * 512, (fb + 1) * 512)
                    nc.tensor.matmul(pt[:, psl], lhsT=ones_sb[:, :128],
                                     rhs=b2_sb[:, fo:fo + 512], start=True, stop=False)
                    for s in range(4):
                        nc.tensor.matmul(
                            pt[:, psl],
                            lhsT=ht_slot(h1, 1, width=BS)[:, s, :],
                            rhs=wih2t_sb[:, s * H4 + fo: s * H4 + fo + 512],
                            start=False, stop=(s == 3))
                nc.vector.tensor_copy(x2[:, half * 1024:(half + 1) * 1024], pt[:])
            # L2 steps
            cur = h2p.tile([128, 4 * HTW], BF16, tag="h2blk")
            nc.vector.tensor_copy(ht_slot(cur, 0), ht_slot(h2_prev, BS))
            for j in range(BS):
                x2s = gp.tile([b, H4], F32, tag="x2s")
                nc.sync.dma_start(out=x2s[:], in_=x2[j * b:(j + 1) * b, :])
                lstm_step(ht_slot(cur, j), ht_slot(cur, j + 1), c2_sb,
                          whh2t_sb, x2s[:], None)
            h2_prev = cur
            # FC: out rows [bk*BS*b : +128] = h2blk.T @ WfcT + bfc
            pf = pfc.tile([128, V], F32, tag="pfc")
            nc.tensor.matmul(pf[:], lhsT=ones_sb[:, :128], rhs=bfc_sb[:],
                             start=True, stop=False)
            for s in range(4):
                nc.tensor.matmul(pf[:], lhsT=ht_slot(cur, 1, width=BS)[:, s, :],
                                 rhs=wfct_sb[:, s * V:(s + 1) * V],
                                 start=False, stop=(s == 3))
            osb = op.tile([128, V], F32, tag="osb")
            nc.vector.tensor_copy(osb[:], pf[:])
            nc.sync.dma_start(
                out=out_ext[bk * BS * b:(bk + 1) * BS * b, :], in_=osb[:])

        l1_block(0)
        for bk in range(1, nb):
            l1_block(bk)
            l2_block(bk - 1)
        l2_block(nb - 1)

    nc.compile()
    return nc


GATE_PERM = np.concatenate([np.arange(0, 512), np.arange(512, 1024),
                            np.arange(1536, 2048), np.arange(1024, 1536)])


def to_bf16(a):
    import ml_dtypes
    return a.astype(ml_dtypes.bfloat16)


def prep_core_inputs(inputs, core, b=8, seq=1024):
    """Host-side prep of one core's input map."""
    x = np.asarray(inputs["x"])          # [seq, 64]
    sl = slice(core * b, (core + 1) * b)
    p = GATE_PERM
    m = {
        "x": np.ascontiguousarray(x[:, sl].T.astype(np.int32)),
        "embt": to_bf16(np.asarray(inputs["emb"]).T),
        "wih1t": to_bf16(np.asarray(inputs["Wih1"])[p].T),
        "b1": (np.asarray(inputs["bih1"]) + np.asarray(inputs["bhh1"]))[p][None].astype(np.float32),
        "whh1t": to_bf16(np.asarray(inputs["Whh1"])[p].T),
        "wih2t": to_bf16(np.asarray(inputs["Wih2"])[p].T),
        "b2": (np.asarray(inputs["bih2"]) + np.asarray(inputs["bhh2"]))[p][None].astype(np.float32),
        "whh2t": to_bf16(np.asarray(inputs["Whh2"])[p].T),
        "wfct": to_bf16(np.asarray(inputs["Wfc"]).T),
        "bfc": np.asarray(inputs["bfc"])[None].astype(np.float32),
        "h0": np.asarray(inputs["h0"])[sl].astype(np.float32),
        "c0": np.asarray(inputs["c0"])[sl].astype(np.float32),
    }
    return m


# ---------------------------------------------------------------------------
# Self-contained entry point: kernel(**inputs) -> np.ndarray [1024, 64, 512]
# ---------------------------------------------------------------------------
SEQ = 1024
NCORES = 8
_B = 64 // NCORES

_nc_cache = {}


def _get_nc():
    if "nc" not in _nc_cache:
        _nc_cache["nc"] = build_nc(seq=SEQ, b=_B, num_devices=NCORES)
    return _nc_cache["nc"]


def kernel(**inputs):
    from concourse.bass_utils import run_bass_kernel_spmd
    nc = _get_nc()
    in_maps = [prep_core_inputs(inputs, core, b=_B, seq=SEQ)
               for core in range(NCORES)]
    res = run_bass_kernel_spmd(nc, in_maps, list(range(NCORES)))
    out = np.stack([np.asarray(res.results[i]["out"]).reshape(SEQ, _B, V)
                    for i in range(NCORES)], axis=1)
    return np.ascontiguousarray(out.reshape(SEQ, 64, V).astype(np.float32))


# revision 4
# speedup vs baseline: 1.2583x; 1.2583x over previous
"""CharLSTM Trainium2 kernel: 2-layer LSTM, SEQ=1024, BATCH=64, H=512, E=256, V=512.

Strategy: data-parallel over batch across 8 cores (B=8 per core), identical SPMD
program. Per core:
  - T1 = emb @ Wih1p.T + b1p  [V, 4H] f32 built on device, stored in DRAM.
  - L1 recurrence: per step, gather T1 rows for x_t into SBUF (X1), inject into
    PSUM via identity matmul, accumulate h1T @ Whh1pT (bf16), activations
    (gate-permuted layout i,f,o,g so sigmoid covers f[0:1536]), cell update,
    h1 transposed via PE into h1T ring (stationary for next step + X2 blocks).
  - Lag-block pipeline: L1 block bk runs alongside X2+L2+FC for block bk-1.
  - FC: out = h2T-block.T @ WfcT + bfc -> DMA to output DRAM.

Gate order permuted from PyTorch (i,f,g,o) to (i,f,o,g).
"""

import numpy as np
from contextlib import ExitStack

import concourse.bass as bass
import concourse.bacc as bacc
import concourse.mybir as mybir
import concourse.tile as tile
from concourse.masks import make_identity

F32 = mybir.dt.float32
BF16 = mybir.dt.bfloat16
I32 = mybir.dt.int32

H = 512
E = 256
V = 512
H4 = 4 * H
BS = 16  # steps per block (overridden via build_nc cfg)


def build_nc(seq=1024, b=8, num_devices=8, cfg=None):
    global BS
    cfg = cfg or {}
    BS = cfg.get("bs", 16)
    PDB = cfg.get("pdb", False)          # psum partition double-buffer for gates
    ACTB = cfg.get("actbufs", 2)
    GPB = cfg.get("gpbufs", 3)
    PTRB = cfg.get("ptrbufs", 1)
    X2BF = cfg.get("x2bf", True)        # stage X2 in bf16
    SIGSPLIT = cfg.get("sigsplit", False)  # per-bank sigmoid
    nb = seq // BS
    nc = bacc.Bacc("TRN2", target_bir_lowering=False, debug=False,
                   num_devices=num_devices)

    dram = nc.dram_tensor
    x_ext = dram("x", [b, seq], I32, kind="ExternalInput").ap()
    embt_ext = dram("embt", [E, V], BF16, kind="ExternalInput").ap()
    wih1t_ext = dram("wih1t", [E, H4], BF16, kind="ExternalInput").ap()
    b1_ext = dram("b1", [1, H4], F32, kind="ExternalInput").ap()
    whh1t_ext = dram("whh1t", [H, H4], BF16, kind="ExternalInput").ap()
    wih2t_ext = dram("wih2t", [H, H4], BF16, kind="ExternalInput").ap()
    b2_ext = dram("b2", [1, H4], F32, kind="ExternalInput").ap()
    whh2t_ext = dram("whh2t", [H, H4], BF16, kind="ExternalInput").ap()
    wfct_ext = dram("wfct", [H, V], BF16, kind="ExternalInput").ap()
    bfc_ext = dram("bfc", [1, V], F32, kind="ExternalInput").ap()
    h0_ext = dram("h0", [b, H], F32, kind="ExternalInput").ap()
    c0_ext = dram("c0", [b, H], F32, kind="ExternalInput").ap()
    out_ext = dram("out", [seq * b, V], F32, kind="ExternalOutput").ap()
    t1_dram = dram("t1buf", [V, H4], F32).ap()

    with ExitStack() as ctx:
        tc = ctx.enter_context(tile.TileContext(nc))
        # ---- persistent pools ----
        wp = ctx.enter_context(tc.tile_pool(name="weights", bufs=1))
        sp = ctx.enter_context(tc.tile_pool(name="state", bufs=1))
        # psum pools: gates 4 banks, tr 1, x2 2, fc 1 -> 8 banks
        pg = ctx.enter_context(tc.tile_pool(name="pg", bufs=1, space="PSUM"))
        ptr = ctx.enter_context(tc.tile_pool(name="ptr", bufs=PTRB, space="PSUM"))
        px2 = ctx.enter_context(tc.tile_pool(name="px2", bufs=1, space="PSUM"))
        pfc = px2  # FC shares the X2 psum pool slot

        # ---- load weights ----
        def load4(ext, name):  # [512, N] dram -> [128, 4*N] sbuf (s-major chunks)
            n = ext.shape[1]
            t = wp.tile([128, 4 * n], BF16, tag=name)
            nc.sync.dma_start(
                out=t[:].rearrange("p (s f) -> p s f", s=4),
                in_=ext.rearrange("(s p) f -> p s f", p=128))
            return t

        whh1t_sb = load4(whh1t_ext, "whh1t")
        wih2t_sb = load4(wih2t_ext, "wih2t")
        whh2t_sb = load4(whh2t_ext, "whh2t")
        wfct_sb = load4(wfct_ext, "wfct")

        b1_sb = wp.tile([1, H4], F32, tag="b1")
        nc.sync.dma_start(out=b1_sb[:], in_=b1_ext[:])
        b2_sb = wp.tile([1, H4], F32, tag="b2")
        nc.sync.dma_start(out=b2_sb[:], in_=b2_ext[:])
        bfc_sb = wp.tile([1, V], F32, tag="bfc")
        nc.sync.dma_start(out=bfc_sb[:], in_=bfc_ext[:])

        ones_sb = wp.tile([1, 128], F32, tag="ones")
        nc.gpsimd.memset(ones_sb[:], 1.0)
        id8f = wp.tile([8, 8], F32, tag="id8f")
        make_identity(nc, id8f[:])
        idbf = wp.tile([b, b], BF16, tag="idbf")
        make_identity(nc, idbf[:])

        x_sb = sp.tile([b, seq], I32, tag="x")
        nc.sync.dma_start(out=x_sb[:], in_=x_ext[:])
        c1_sb = sp.tile([b, H], F32, tag="c1")
        nc.sync.dma_start(out=c1_sb[:], in_=c0_ext[:])
        c2_sb = sp.tile([b, H], F32, tag="c2")
        nc.sync.dma_start(out=c2_sb[:], in_=c0_ext[:])

        # ---- T1 = emb @ Wih1p.T + b1p ----
        with tc.tile_pool(name="t1tmp", bufs=1) as t1tmp, \
             tc.tile_pool(name="t1cp", bufs=2) as t1cp:
            embt_sb = t1tmp.tile([128, 2 * V], BF16, tag="embt")
            nc.sync.dma_start(
                out=embt_sb[:].rearrange("p (s f) -> p s f", s=2),
                in_=embt_ext.rearrange("(s p) f -> p s f", p=128))
            wih1t_sb = t1tmp.tile([128, 2 * H4], BF16, tag="wih1t")
            nc.sync.dma_start(
                out=wih1t_sb[:].rearrange("p (s f) -> p s f", s=2),
                in_=wih1t_ext.rearrange("(s p) f -> p s f", p=128))
            for vc in range(4):
                pt = pg.tile([128, H4], F32, tag="pg")
                for fb in range(4):
                    fsl = slice(fb * 512, (fb + 1) * 512)
                    nc.tensor.matmul(pt[:, fsl], lhsT=ones_sb[:, :128],
                                     rhs=b1_sb[:, fsl], start=True, stop=False)
                    for ks in range(2):
                        nc.tensor.matmul(
                            pt[:, fsl],
                            lhsT=embt_sb[:, ks * V + vc * 128: ks * V + (vc + 1) * 128],
                            rhs=wih1t_sb[:, ks * H4 + fb * 512: ks * H4 + (fb + 1) * 512],
                            start=False, stop=(ks == 1))
                t1s = t1cp.tile([128, H4], F32, tag="t1s")
                nc.vector.tensor_copy(t1s[:], pt[:])
                nc.sync.dma_start(out=t1_dram[vc * 128:(vc + 1) * 128, :], in_=t1s[:])

        # ---- per-step pools (opened after t1tmp frees its space) ----
        h1p = ctx.enter_context(tc.tile_pool(name="h1t", bufs=3))
        h2p = ctx.enter_context(tc.tile_pool(name="h2t", bufs=2))
        gp = ctx.enter_context(tc.tile_pool(name="gath", bufs=3))
        ap_ = ctx.enter_context(tc.tile_pool(name="act", bufs=ACTB))
        x2p = ctx.enter_context(tc.tile_pool(name="x2sb", bufs=2))
        op = ctx.enter_context(tc.tile_pool(name="outsb", bufs=2))

        # ---- initial h transposes ----
        def transpose_h(hbf, dst128x4b):
            """hbf [b, 512] bf16 -> dst [128, 4*b] bf16 (s-major)."""
            ptile = ptr.tile([128, 4 * b], BF16, tag="ptr")
            for s in range(4):
                nc.tensor.transpose(
                    out=ptile[:, s * b:(s + 1) * b],
                    in_=hbf[:, s * 128:(s + 1) * 128],
                    identity=idbf[:])
            nc.vector.tensor_copy(dst128x4b, ptile[:])

        # h1t block tiles: [128, 4 * (BS+1) * b]; slot j in 0..BS, s-plane stride (BS+1)*b
        HTW = (BS + 1) * b  # f-width per s-plane

        def ht_slot(t_tile, j, width=1):
            return t_tile[:].rearrange("p (s q) -> p s q", s=4)[:, :, j * b:(j + width) * b]

        h0_f32 = ap_.tile([b, H], F32, tag="t1_")
        nc.sync.dma_start(out=h0_f32[:], in_=h0_ext[:])
        h0_bf = ap_.tile([b, H], BF16, tag="h1bf")
        nc.vector.tensor_copy(h0_bf[:], h0_f32[:])

        h1blk = h1p.tile([128, 4 * HTW], BF16, tag="h1blk")
        transpose_h(h0_bf, ht_slot(h1blk, BS))
        h2blk = h2p.tile([128, 4 * HTW], BF16, tag="h2blk")
        transpose_h(h0_bf, ht_slot(h2blk, BS))

        def lstm_step(ht_prev_ap, ht_out_ap, c_sb, whht_sb, inject_rhs, pbase=0):
            """One LSTM step. ht_prev_ap: [128, 4, b] stationary (s-plane view).
            inject_rhs: f32 [8, 2048]-like AP for X contribution, or tile to gather into.
            gather_idx: if not None, (idx_ap) -> gather T1 rows into inject tile first.
            Returns new h bf16 tile [b, 512]."""
            gfull = pg.tile([(40 if PDB else b), H4], F32, tag="pg", name="gfull")
            gt = gfull[pbase:pbase + b, :]
            inj_id = id8f if inject_rhs.dtype == F32 else idbf
            for fb in range(4):
                fsl = slice(fb * 512, (fb + 1) * 512)
                nc.tensor.matmul(gt[:, fsl], lhsT=inj_id[:b, :b],
                                 rhs=inject_rhs[:, fsl], start=True, stop=False)
                for s in range(4):
                    nc.tensor.matmul(
                        gt[:, fsl], lhsT=ht_prev_ap[:, s, :],
                        rhs=whht_sb[:, s * H4 + fb * 512: s * H4 + (fb + 1) * 512],
                        start=False, stop=(s == 3))
            sig = ap_.tile([b, 3 * H], F32, tag="sig")
            if SIGSPLIT:
                for sb_ in range(3):
                    nc.scalar.activation(sig[:, sb_ * H:(sb_ + 1) * H],
                                         gt[:, sb_ * H:(sb_ + 1) * H],
                                         mybir.ActivationFunctionType.Sigmoid)
            else:
                nc.scalar.activation(sig[:], gt[:, 0:3 * H],
                                     mybir.ActivationFunctionType.Sigmoid)
            tg = ap_.tile([b, H], F32, tag="tg")
            nc.scalar.activation(tg[:], gt[:, 3 * H:4 * H],
                                 mybir.ActivationFunctionType.Tanh)
            t1_ = ap_.tile([b, H], F32, tag="t1_")
            nc.vector.tensor_mul(t1_[:], sig[:, H:2 * H], c_sb[:])      # f*c
            t2_ = ap_.tile([b, H], F32, tag="t2_")
            nc.vector.tensor_mul(t2_[:], sig[:, 0:H], tg[:])            # i*g
            nc.vector.tensor_add(c_sb[:], t1_[:], t2_[:])               # c = f*c+i*g
            tc_ = ap_.tile([b, H], F32, tag="tg")
            nc.scalar.activation(tc_[:], c_sb[:],
                                 mybir.ActivationFunctionType.Tanh)
            hbf = ap_.tile([b, H], BF16, tag="h1bf")
            nc.vector.tensor_mul(hbf[:], sig[:, 2 * H:3 * H], tc_[:])   # h = o*tanh(c)
            transpose_h(hbf, ht_out_ap)
            return hbf

        h1_tiles = {}   # bk -> h1blk tile
        h1_tiles[-1] = h1blk
        h2_prev = h2blk

        def l1_start(bk):
            prev = h1_tiles[bk - 1]
            cur = h1p.tile([128, 4 * HTW], BF16, tag="h1blk")
            nc.vector.tensor_copy(ht_slot(cur, 0), ht_slot(prev, BS))
            h1_tiles[bk] = cur
            if bk >= 3 and (bk - 3) in h1_tiles:
                del h1_tiles[bk - 3]
            return cur

        def l1_step(bk, j, cur):
            t = bk * BS + j
            x1 = gp.tile([b, H4], F32, tag="x1", bufs=GPB)
            nc.gpsimd.indirect_dma_start(
                out=x1[:], out_offset=None, in_=t1_dram[:],
                in_offset=bass.IndirectOffsetOnAxis(ap=x_sb[:, t:t + 1], axis=0))
            lstm_step(ht_slot(cur, j), ht_slot(cur, j + 1), c1_sb,
                      whh1t_sb, x1[:], pbase=(32 if (PDB and t % 2) else 0))

        def x2_block(bk):
            h1 = h1_tiles[bk]
            x2 = x2p.tile([BS * b, H4], (BF16 if X2BF else F32), tag="x2")
            for half in range(2):
                pt = px2.tile([128, 1024], F32, tag="px2")
                for fb in range(2):
                    fo = half * 1024 + fb * 512
                    psl = slice(fb * 512, (fb + 1) * 512)
                    nc.tensor.matmul(pt[:, psl], lhsT=ones_sb[:, :128],
                                     rhs=b2_sb[:, fo:fo + 512], start=True, stop=False)
                    for s in range(4):
                        nc.tensor.matmul(
                            pt[:, psl],
                            lhsT=ht_slot(h1, 1, width=BS)[:, s, :],
                            rhs=wih2t_sb[:, s * H4 + fo: s * H4 + fo + 512],
                            start=False, stop=(s == 3))
                nc.vector.tensor_copy(x2[:, half * 1024:(half + 1) * 1024], pt[:])
            return x2

        def l2_start():
            cur = h2p.tile([128, 4 * HTW], BF16, tag="h2blk")
            nc.vector.tensor_copy(ht_slot(cur, 0), ht_slot(h2_prev, BS))
            return cur

        def l2_step(j, cur, x2):
            x2s = gp.tile([b, H4], (BF16 if X2BF else F32), tag="x2s", bufs=(GPB if X2BF else 3))
            nc.sync.dma_start(out=x2s[:], in_=x2[j * b:(j + 1) * b, :])
            lstm_step(ht_slot(cur, j), ht_slot(cur, j + 1), c2_sb,
                      whh2t_sb, x2s[:], pbase=(32 if (PDB and j % 2) else 0))

        def fc_block(bk, cur):
            pf_full = pfc.tile([128, 1024], F32, tag="px2", name="pf_full")
            pf = pf_full[:, :V]
            nc.tensor.matmul(pf[:], lhsT=ones_sb[:, :128], rhs=bfc_sb[:],
                             start=True, stop=False)
            for s in range(4):
                nc.tensor.matmul(pf[:], lhsT=ht_slot(cur, 1, width=BS)[:, s, :],
                                 rhs=wfct_sb[:, s * V:(s + 1) * V],
                                 start=False, stop=(s == 3))
            osb = op.tile([128, V], F32, tag="osb")
            nc.vector.tensor_copy(osb[:], pf[:])
            nc.sync.dma_start(
                out=out_ext[bk * BS * b:(bk + 1) * BS * b, :], in_=osb[:])

        # prologue: L1 block 0 alone
        cur1 = l1_start(0)
        for j in range(BS):
            l1_step(0, j, cur1)
        # steady state: interleave L1(bk) steps with L2(bk-1) steps
        for bk in range(1, nb):
            x2 = x2_block(bk - 1)
            cur1 = l1_start(bk)
            cur2 = l2_start()
            for j in range(BS):
                l1_step(bk, j, cur1)
                l2_step(j, cur2, x2)
            h2_prev = cur2
            fc_block(bk - 1, cur2)
        # epilogue: L2 block nb-1
        x2 = x2_block(nb - 1)
        cur2 = l2_start()
        for j in range(BS):
            l2_step(j, cur2, x2)
        h2_prev = cur2
        fc_block(nb - 1, cur2)

    nc.compile()
    return nc


GATE_PERM = np.concatenate([np.arange(0, 512), np.arange(512, 1024),
                            np.arange(1536, 2048), np.arange(1024, 1536)])


def to_bf16(a):
    import ml_dtypes
    return a.astype(ml_dtypes.bfloat16)


def prep_core_inputs(inputs, core, b=8, seq=1024):
    """Host-side prep of one core's input map."""
    x = np.asarray(inputs["x"])          # [seq, 64]
    sl = slice(core * b, (core + 1) * b)
    p = GATE_PERM
    m = {
        "x": np.ascontiguousarray(x[:, sl].T.astype(np.int32)),
        "embt": to_bf16(np.asarray(inputs["emb"]).T),
        "wih1t": to_bf16(np.asarray(inputs["Wih1"])[p].T),
        "b1": (np.asarray(inputs["bih1"]) + np.asarray(inputs["bhh1"]))[p][None].astype(np.float32),
        "whh1t": to_bf16(np.asarray(inputs["Whh1"])[p].T),
        "wih2t": to_bf16(np.asarray(inputs["Wih2"])[p].T),
        "b2": (np.asarray(inputs["bih2"]) + np.asarray(inputs["bhh2"]))[p][None].astype(np.float32),
        "whh2t": to_bf16(np.asarray(inputs["Whh2"])[p].T),
        "wfct": to_bf16(np.asarray(inputs["Wfc"]).T),
        "bfc": np.asarray(inputs["bfc"])[None].astype(np.float32),
        "h0": np.asarray(inputs["h0"])[sl].astype(np.float32),
        "c0": np.asarray(inputs["c0"])[sl].astype(np.float32),
    }
    return m


# ---------------------------------------------------------------------------
# Self-contained entry point: kernel(**inputs) -> np.ndarray [1024, 64, 512]
# ---------------------------------------------------------------------------
SEQ = 1024
NCORES = 8
_B = 64 // NCORES

_cache = {}


def _get_fn():
    """Build program once; return (jitted_fn, in_names, out_names, out_avals)."""
    if "fn" in _cache:
        return _cache["fn"]
    import jax
    from jax.sharding import Mesh, PartitionSpec
    from jax.experimental.shard_map import shard_map
    from concourse.bass2jax import (_bass_exec_p, install_neuronx_cc_hook,
                                    partition_id_tensor)
    install_neuronx_cc_hook()
    nc = build_nc(seq=SEQ, b=_B, num_devices=NCORES)

    partition_name = nc.partition_id_tensor.name if nc.partition_id_tensor else None
    in_names, out_names, out_avals, zero_outs = [], [], [], []
    for alloc in nc.m.functions[0].allocations:
        if not isinstance(alloc, mybir.MemoryLocationSet):
            continue
        name = alloc.memorylocations[0].name
        if alloc.kind == "ExternalInput":
            if name != partition_name:
                in_names.append(name)
        elif alloc.kind == "ExternalOutput":
            shape = tuple(alloc.tensor_shape)
            dt = mybir.dt.np(alloc.dtype)
            out_names.append(name)
            out_avals.append(jax.core.ShapedArray(shape, dt))
            zero_outs.append(np.zeros(shape, dt))
    n_params = len(in_names)
    in_names_full = in_names + out_names + ([partition_name] if partition_name else [])

    def _body(*args):
        operands = list(args)
        if partition_name is not None:
            operands.append(partition_id_tensor())
        return tuple(_bass_exec_p.bind(
            *operands, out_avals=tuple(out_avals), in_names=tuple(in_names_full),
            out_names=tuple(out_names), lowering_input_output_aliases=(),
            sim_require_finite=True, sim_require_nnan=True, nc=nc))

    devices = jax.devices()[:NCORES]
    mesh = Mesh(np.asarray(devices), ("core",))
    nio = n_params + len(out_names)
    fn = jax.jit(shard_map(_body, mesh=mesh,
                           in_specs=(PartitionSpec("core"),) * nio,
                           out_specs=(PartitionSpec("core"),) * len(out_names),
                           check_rep=False), keep_unused=True)
    _cache["fn"] = (fn, mesh, in_names, out_names, out_avals, zero_outs)
    return _cache["fn"]


def kernel(**inputs):
    import jax
    from jax.sharding import NamedSharding, PartitionSpec
    fn, mesh, in_names, out_names, out_avals, zero_outs = _get_fn()
    in_maps = [prep_core_inputs(inputs, core, b=_B, seq=SEQ)
               for core in range(NCORES)]
    concat_in = [np.concatenate([np.asarray(in_maps[c][n]) for c in range(NCORES)],
                                axis=0) for n in in_names]
    concat_zeros = [np.zeros((NCORES * z.shape[0], *z.shape[1:]), z.dtype)
                    for z in zero_outs]
    sh = NamedSharding(mesh, PartitionSpec("core"))
    dev_args = [jax.device_put(a, sh) for a in concat_in + concat_zeros]
    outs = fn(*dev_args)
    oi = out_names.index("out")
    res = np.asarray(outs[oi]).reshape(NCORES, SEQ, _B, V)
    out = np.stack([res[i] for i in range(NCORES)], axis=1)  # [SEQ, core, b, V]
    return np.ascontiguousarray(out.reshape(SEQ, 64, V).astype(np.float32))
